# revision 26
# baseline (speedup 1.0000x reference)
"""Trainium2 Bass kernel for the LZD encoder (gnn_message_passing).

Strategy (data parallel over n_seq, 1 sequence per core, 8 cores):
  - h[seq] lives in DRAM as bf16 [4104, 1024]: row 4096 is a garbage sink
    (so every gather/scatter call has a fully static index count) and rows
    4097..4103 guard against any end-of-tensor DMA overreach.
  - Leaf phase: the char-embedding scatter-add is reformulated as a dense
    matmul  h_init = Count @ emb  where Count[pos, char] is computed on the
    host from the (index-only) char event lists.
  - Each of the 12 levels: transposed dma_gather of left/right operand rows
    (X^T tiles, feature-major), dense PE matmul against resident W, tanh on
    the scalar engine, per-column dma_scatter_add back into h.  The host
    pre-sorts each level's events into 128-slot columns such that no column
    contains a duplicate destination position (the hardware scatter-add is
    not atomic across DMA engines).

  Level-boundary pipelining (the main optimization over the naive order):
  the conservative whole-tensor hazard tracking on h serializes any gather
  enqueued after a scatter.  Each level's events are therefore split into
  HEAD columns (events whose BOTH operand rows are untouched by the
  previous level's scatters) and TAIL columns (the rest).  HEAD gathers for
  level l+1 are enqueued BEFORE level l's scatters in program order - the
  tracker then only adds a harmless write-after-read edge - so they execute
  during level l's compute and the PE rolls across the level boundary with
  no bubble while level l+1's TAIL gathers run behind it.
"""
import numpy as np
import ml_dtypes

from concourse import bass, bacc, tile, mybir
from concourse import bass_utils

BF16 = ml_dtypes.bfloat16
N_SEQ, MAX_LEN, H_DIM, N_CHAR = 8, 4096, 1024, 256
L_LEVELS = 12
SINK = MAX_LEN  # garbage row index in the [4104, 1024] h tensor
N_CORES = 8
HEAD_CAP = 5  # max head columns per level

_cache = {}
_last_results = None


def _ramp_sizes(m):
    """Gather-group column sizes with a small-first ramp: [1, 1, 2, 2, ...].
    A group's fused left+right gather is 2*size*128 indices and must stay
    <= 512."""
    sizes = []
    rem = m
    for _ in (0, 1):
        if rem <= 0:
            break
        sizes.append(1)
        rem -= 1
    while rem > 0:
        t = min(2, rem)
        sizes.append(t)
        rem -= t
    return sizes


def _flat_sizes(m):
    """Gather-group column sizes, max-size groups: [2, 2, ..., (1)]."""
    return [2] * (m // 2) + ([1] if m % 2 else [])


def _kind_sizes(S_list, nh_list, kind, l):
    if kind == "h":
        return _flat_sizes(nh_list[l])
    if l == 0:
        return _ramp_sizes(S_list[0] // 128)
    return _flat_sizes(S_list[l] // 128 - nh_list[l])


def _schedule(S_list, nh_list):
    """Gather enqueue order shared by host packing and device build.
    Per-group entries (kind, level, g, glen, col0): tail(0), head(1), then
    for each level l>=1: tail(l) groups 0-2, head(l+1), tail(l) rest.
    Returns the flat per-group list."""
    out = []

    def emit(kind, l, frm=0, until=None):
        if kind == "h" and not nh_list[l]:
            return
        sizes = _kind_sizes(S_list, nh_list, kind, l)
        col0 = 0 if kind == "h" else nh_list[l]
        col0 += sum(sizes[:frm])
        for g, glen in list(enumerate(sizes))[frm:until]:
            out.append((kind, l, g, glen, col0))
            col0 += glen

    emit("t", 0)
    emit("h", 1)
    for l in range(1, L_LEVELS):
        emit("t", l, until=3)
        if l + 1 < L_LEVELS:
            emit("h", l + 1)
        emit("t", l, frm=3)
    return out


# ------------------------------------------------------------------ host prep

def _wrap_idx(idx):
    """int16 index layout for dma_gather/dma_scatter_add: arr[p, s] =
    idx[s*16 + p]; result [16, len/16]."""
    idx = np.asarray(idx, dtype=np.int16)
    assert idx.size % 16 == 0
    return idx.reshape(-1, 16).T


def _place_level(pos, elig, nh, ncol):
    """Assign each event (with scatter destination pos[i]) a slot in
    [0, ncol*128) such that no 128-slot column holds two events with the
    same destination.  Only eligible events (operands untouched by the
    previous level) may occupy the nh head columns (cols 0..nh-1).
    Returns slot array (or None if infeasible)."""
    n = len(pos)
    slots = np.full(n, -1, dtype=np.int64)
    col_fill = np.zeros(ncol, dtype=np.int64)
    col_has = [set() for _ in range(ncol)]
    uniq, inv, counts = np.unique(pos, return_inverse=True, return_counts=True)
    mult = counts[inv]

    def try_place(i, c0, c1):
        p = pos[i]
        best = -1
        for c in range(c0, c1):
            if col_fill[c] < 128 and p not in col_has[c]:
                if best < 0 or col_fill[c] < col_fill[best]:
                    best = c
        if best < 0:
            return False
        slots[i] = best * 128 + col_fill[best]
        col_fill[best] += 1
        col_has[best].add(p)
        return True

    if nh > 0:
        eidx = np.nonzero(elig)[0]
        order = eidx[np.argsort(-mult[eidx], kind="stable")]
        placed = 0
        for i in order:
            if placed >= nh * 128:
                break
            if try_place(i, 0, nh):
                placed += 1
    rest = np.nonzero(slots < 0)[0]
    order = rest[np.argsort(-mult[rest], kind="stable")]
    for i in order:
        if not try_place(i, nh, ncol):
            return None
    return slots


def _prep(inputs):
    ci_seq = np.asarray(inputs["char_i_seq"]).astype(np.int64)
    ci_pos = np.asarray(inputs["char_i_pos"]).astype(np.int64)
    c_ids = np.asarray(inputs["char_ids"]).astype(np.int64)
    gi_seq = np.asarray(inputs["grp_i_seq"]).astype(np.int64)
    gi_first = np.asarray(inputs["grp_i_first"]).astype(np.int64)
    gi_second = np.asarray(inputs["grp_i_second"]).astype(np.int64)
    gi_pos = np.asarray(inputs["grp_i_pos"]).astype(np.int64)
    emb = np.asarray(inputs["emb_char"]).astype(np.float32)
    W = np.asarray(inputs["W"]).astype(np.float32)
    b = np.asarray(inputs["b"]).astype(np.float32)

    has_bias = bool(np.any(b != 0.0))
    w_bf = np.ascontiguousarray(W.astype(BF16))
    b_bf = np.ascontiguousarray(
        np.broadcast_to(b.reshape(1, H_DIM), (128, H_DIM)).astype(np.float32))
    emb_bf = np.ascontiguousarray(emb.astype(BF16))

    # leaf counts: Count[pos, char] per core -> ship transposed [char, pos]
    cnts = []
    for s in range(N_CORES):
        m = ci_seq == s
        cnt = np.zeros((MAX_LEN, N_CHAR), dtype=np.float64)
        np.add.at(cnt, (ci_pos[m], c_ids[m]), 1.0)
        cnts.append(np.ascontiguousarray(cnt.T.astype(BF16)))

    # per-core per-level event lists
    ev_ls = [[np.nonzero(gi_seq[l] == s)[0] for s in range(N_CORES)]
             for l in range(L_LEVELS)]
    # rows written by each level on each core (real scatter destinations)
    writ = [[None] * N_CORES for _ in range(L_LEVELS)]
    for l in range(L_LEVELS):
        for s in range(N_CORES):
            w = np.zeros(MAX_LEN, dtype=bool)
            w[gi_pos[l][ev_ls[l][s]]] = True
            writ[l][s] = w
    # head eligibility: both operand rows untouched by the previous level
    elig = [[None] * N_CORES for _ in range(L_LEVELS)]
    nh_list = [0] * L_LEVELS
    for l in range(1, L_LEVELS):
        cnts_e = []
        for s in range(N_CORES):
            ev = ev_ls[l][s]
            pw = writ[l - 1][s]
            e = (~pw[gi_first[l][ev]]) & (~pw[gi_second[l][ev]])
            elig[l][s] = e
            cnts_e.append(int(e.sum()))
        nh_list[l] = min(HEAD_CAP, min(cnts_e) // 128)

    S_list = []
    for l in range(L_LEVELS):
        mx = max(len(ev_ls[l][s]) for s in range(N_CORES))
        S_list.append(max(((mx + 127) // 128) * 128, 1152))

    # placement with retry on infeasibility
    g1_all = [[None] * L_LEVELS for _ in range(N_CORES)]
    g2_all = [[None] * L_LEVELS for _ in range(N_CORES)]
    sc_all = [[None] * L_LEVELS for _ in range(N_CORES)]
    for l in range(L_LEVELS):
        while True:
            S = S_list[l]
            ncol = S // 128
            nh = min(nh_list[l], max(0, ncol - 6))
            ok = True
            rows = []
            for s in range(N_CORES):
                ev = ev_ls[l][s]
                pos = gi_pos[l][ev]
                e = elig[l][s] if l >= 1 else np.zeros(len(ev), dtype=bool)
                slots = _place_level(pos, e, nh, ncol)
                if slots is None:
                    S_list[l] = S + 128
                    ok = False
                    break
                g1 = np.full(S, SINK, dtype=np.int64)
                g2 = np.full(S, SINK, dtype=np.int64)
                sc = np.full(S, SINK, dtype=np.int64)
                g1[slots] = gi_first[l][ev]
                g2[slots] = gi_second[l][ev]
                sc[slots] = pos
                # invariant: head-column rows untouched by previous level
                if l >= 1 and nh > 0:
                    pw = writ[l - 1][s]
                    hr = np.concatenate([g1[: nh * 128], g2[: nh * 128]])
                    hr = hr[hr != SINK]
                    assert not pw[hr].any(), "head eligibility violated"
                rows.append((g1, g2, sc))
            if ok:
                nh_list[l] = nh
                break
        for s, (g1, g2, sc) in enumerate(rows):
            g1_all[s][l] = g1
            g2_all[s][l] = g2
            sc_all[s][l] = sc

    sched = _schedule(S_list, nh_list)

    def pack_gc(s):
        chunks = []
        for kind, l, g, glen, col0 in sched:
            g1, g2 = g1_all[s][l], g2_all[s][l]
            chunks.append(g1[col0 * 128: (col0 + glen) * 128])
            chunks.append(g2[col0 * 128: (col0 + glen) * 128])
        flat = np.concatenate(chunks)
        return np.ascontiguousarray(np.tile(_wrap_idx(flat), (8, 1)))

    def pack_sc(s):
        flat = np.concatenate(sc_all[s])
        return np.ascontiguousarray(np.tile(_wrap_idx(flat), (8, 1)))

    in_maps = []
    for s in range(N_CORES):
        in_maps.append({
            "w": w_bf,
            "b": b_bf,
            "emb": emb_bf,
            "cnt": cnts[s],
            "gc": pack_gc(s),
            "sc": pack_sc(s),
        })
    return tuple(S_list), tuple(nh_list), has_bias, in_maps


# -------------------------------------------------------------- device program

def _build(S_list, nh_list, has_bias):
    tot16 = sum(S_list) // 16
    sched = _schedule(S_list, nh_list)
    nc = bacc.Bacc("TRN2", target_bir_lowering=False, debug=False,
                   num_swdge_queues=4)
    w_in = nc.dram_tensor("w", [2 * H_DIM, H_DIM], mybir.dt.bfloat16,
                          kind="ExternalInput")
    b_in = nc.dram_tensor("b", [128, H_DIM], mybir.dt.float32,
                          kind="ExternalInput")
    emb_in = nc.dram_tensor("emb", [N_CHAR, H_DIM], mybir.dt.bfloat16,
                            kind="ExternalInput")
    cnt_in = nc.dram_tensor("cnt", [N_CHAR, MAX_LEN], mybir.dt.bfloat16,
                            kind="ExternalInput")
    gc_in = nc.dram_tensor("gc", [128, 2 * tot16], mybir.dt.int16,
                           kind="ExternalInput")
    sc_in = nc.dram_tensor("sc", [128, tot16], mybir.dt.int16,
                           kind="ExternalInput")
    h = nc.dram_tensor("h", [MAX_LEN + 8, H_DIM], mybir.dt.bfloat16,
                       kind="ExternalOutput")

    # cross-queue ordering fences: with gathers and scatters on different
    # swdge queues, FIFO order no longer implies completion order.  Explicit
    # dependency edges (lowered by the tile framework onto its per-queue DMA
    # completion counters) restore scatter->gather RAW and gather->scatter
    # WAR ordering at the fence points.
    from concourse.tile_rust import add_dep_helper
    fence_gathers = []
    level_scatters = []

    with tile.TileContext(nc) as tc:
        with (
            tc.tile_pool(name="const", bufs=1) as const,
            tc.tile_pool(name="work", bufs=1) as work,
            tc.tile_pool(name="outp", bufs=6) as outp,
            tc.tile_pool(name="stage", bufs=3) as stage,
            tc.tile_pool(name="ps", bufs=2, space=bass.MemorySpace.PSUM) as ps,
        ):
            # leaf inputs on the sync queue (they gate everything downstream);
            # cnt split in 4 chunks so the first leaf matmul starts early
            emb_sb = const.tile([128, 2, H_DIM], mybir.dt.bfloat16)
            for kc in range(2):
                nc.sync.dma_start(emb_sb[:, kc, :],
                                  emb_in.ap()[kc * 128: (kc + 1) * 128])
            cnt_sb = const.tile([128, 2, MAX_LEN], mybir.dt.bfloat16)
            half = MAX_LEN // 2
            for hf in range(2):
                for kc in range(2):
                    eng = nc.sync if kc == 0 else nc.scalar
                    eng.dma_start(
                        cnt_sb[:, kc, hf * half: (hf + 1) * half],
                        cnt_in.ap()[kc * 128: (kc + 1) * 128,
                                    hf * half: (hf + 1) * half])
            zrow = const.tile([8, H_DIM], mybir.dt.bfloat16)
            nc.gpsimd.memset(zrow[:], 0.0)
            nc.sync.dma_start(h.ap()[SINK: SINK + 8], zrow[:])
            # remaining consts on the scalar queue so they don't stall the
            # leaf h-writes behind them on the sync queue
            gc_sb = const.tile([128, 2 * tot16], mybir.dt.int16)
            nc.scalar.dma_start(gc_sb[:], gc_in.ap())
            sc_sb = const.tile([128, tot16], mybir.dt.int16)
            nc.scalar.dma_start(sc_sb[:], sc_in.ap())
            bias_sb = const.tile([128, H_DIM], mybir.dt.float32)
            nc.scalar.dma_start(bias_sb[:], b_in.ap())
            w_sb = const.tile([128, 16, H_DIM], mybir.dt.bfloat16)
            nc.scalar.dma_start(w_sb[:], w_in.ap().rearrange("(k p) h -> p k h", p=128))

            # ---- leaf phase: h[t*128:(t+1)*128] = Count @ emb
            for t in range(MAX_LEN // 128):
                p0 = ps.tile([128, 512], mybir.dt.float32)
                p1 = ps.tile([128, 512], mybir.dt.float32)
                for kc in range(2):
                    lhsT = cnt_sb[:, kc, t * 128: (t + 1) * 128]
                    nc.tensor.matmul(p0[:], lhsT, emb_sb[:, kc, 0:512],
                                     start=kc == 0, stop=kc == 1)
                    nc.tensor.matmul(p1[:], lhsT, emb_sb[:, kc, 512:1024],
                                     start=kc == 0, stop=kc == 1)
                hst = stage.tile([128, H_DIM], mybir.dt.bfloat16)
                nc.scalar.copy(hst[:, 0:512], p0[:])
                nc.vector.tensor_copy(hst[:, 512:1024], p1[:])
                nc.sync.dma_start(h.ap()[t * 128: (t + 1) * 128], hst[:])

            # ---- level phases
            ncols = [S // 128 for S in S_list]
            sc_offs = [0] * L_LEVELS
            for l in range(1, L_LEVELS):
                sc_offs[l] = sc_offs[l - 1] + ncols[l - 1] * 8

            gc_pos = [0]  # running offset into gc_sb, int16 columns
            prev_sc = [None]
            x_tiles = {}  # (kind, level) -> list of (tile, glen)

            # Lane/queue consistency: the tile scheduler tracks Pool-DMA
            # completions on lanes DMASW[k %% 8] and its cross-engine waits
            # assume each lane's +16s arrive in call order, which only holds
            # if all calls on a lane ride one queue.  queue = k %% 4 gives
            # lane k%%8 <-> queue k%%4.  (_check_lanes verifies.)
            pool_dma_k = [0]

            def next_queue():
                q = pool_dma_k[0] % 4
                pool_dma_k[0] += 1
                return q

            def enqueue_kind(kind, l, frm=0, until=None):
                if kind == "h" and not nh_list[l]:
                    return
                sizes = _kind_sizes(S_list, nh_list, kind, l)
                sizes_sel = list(enumerate(sizes))[frm:until]
                # tags are per (glen, instance-of-that-glen) so a tag's
                # buffer shape never changes across levels
                n_inst = {}
                for g2_ in range(frm):
                    gl_ = sizes[g2_]
                    n_inst[gl_] = n_inst.get(gl_, 0) + 1
                for g, glen in sizes_sel:
                    i = n_inst.get(glen, 0)
                    n_inst[glen] = i + 1
                    tag = (f"xh{glen}_{i}_{l % 2}" if kind == "h"
                           else f"xt{glen}_{i}")
                    x12 = work.tile([128, 8, 2 * glen * 128],
                                    mybir.dt.bfloat16, tag=tag)
                    n = 2 * glen * 128
                    o = gc_pos[0]
                    q = next_queue()
                    gi = nc.gpsimd.dma_gather(
                        x12[:], h.ap(), gc_sb[:, o: o + glen * 16],
                        n, n, H_DIM, transpose=True, queue_num=q,
                    )
                    fence_gathers.append(gi)
                    gc_pos[0] += glen * 16
                    x_tiles.setdefault((kind, l), []).append((x12, glen))

            def col_lookup(l, j):
                """column j of level l -> (x tile, glen, jj within group)"""
                nh = nh_list[l]
                kind, jl = ("h", j) if j < nh else ("t", j - nh)
                tiles = x_tiles[(kind, l)]
                for x12, glen in tiles:
                    if jl < glen:
                        return x12, glen, jl
                    jl -= glen
                raise AssertionError

            enqueue_kind("t", 0)  # level-0 gathers (after leaf writes)
            enqueue_kind("h", 1)

            for l in range(L_LEVELS):
                for j in range(ncols[l]):
                    x12, glen, jj = col_lookup(l, j)
                    p = ps.tile([128, H_DIM], mybir.dt.float32)
                    for k in range(16):
                        co = jj if k < 8 else glen + jj
                        lhsT = x12[:, k % 8, co * 128: (co + 1) * 128]
                        first, last = k == 0, k == 15
                        nc.tensor.matmul(p[:, 0:512], lhsT, w_sb[:, k, 0:512],
                                         start=first, stop=last)
                        nc.tensor.matmul(p[:, 512:1024], lhsT,
                                         w_sb[:, k, 512:1024],
                                         start=first, stop=last)
                    if has_bias:
                        nc.vector.tensor_add(p[:], p[:], bias_sb[:])
                    outt = outp.tile([128, 1, H_DIM], mybir.dt.bfloat16,
                                     tag="out")
                    nc.scalar.activation(outt[:, 0, :], p[:],
                                         mybir.ActivationFunctionType.Tanh)
                    sci = nc.gpsimd.dma_scatter_add(
                        h.ap(), outt[:],
                        sc_sb[:, sc_offs[l] + j * 8: sc_offs[l] + (j + 1) * 8],
                        128, 128, H_DIM, queue_num=next_queue())
                    if j == 0:
                        # WAR fence: all gathers enqueued so far (incl. this
                        # level's tail and the next level's head) must have
                        # read h before this level starts overwriting it
                        for g_ in fence_gathers:
                            add_dep_helper(sci.ins, g_.ins,
                                           reason="war: scatter after gathers")
                        fence_gathers.clear()
                    if prev_sc[0] is not None:
                        # explicit WAW chain: scatter-adds to the same row in
                        # different columns must not race (they now ride
                        # different queues, so queue FIFO no longer orders
                        # them)
                        add_dep_helper(sci.ins, prev_sc[0].ins,
                                       reason="waw: scatter chain")
                    prev_sc[0] = sci
                    level_scatters.append(sci)
                if l + 1 < L_LEVELS:
                    # RAW fence: tail gathers of level l+1 read rows this
                    # level's scatters may have written; edge onto the first
                    # tail gather (later ones follow in engine order).  The
                    # head gathers of level l+2 are slotted in after the
                    # third tail group so they trigger (and land) early in
                    # level l+1 instead of at its end.
                    n_before = len(fence_gathers)
                    enqueue_kind("t", l + 1, until=3)
                    if l + 2 < L_LEVELS:
                        enqueue_kind("h", l + 2)
                    enqueue_kind("t", l + 1, frm=3)
                    first_tail = fence_gathers[n_before]
                    for s_ in level_scatters:
                        add_dep_helper(first_tail.ins, s_.ins,
                                       reason="raw: gather after scatters")
                    level_scatters.clear()
                    x_tiles.pop(("t", l), None)
                    x_tiles.pop(("h", l), None)

    nc.compile()
    _check_lanes(nc)
    return nc


def _check_lanes(nc):
    """Every Pool-engine DMA's DMASW lane must only ever be updated from one
    swdge queue (lane k%8 <-> queue k%4); a violation silently corrupts the
    tile scheduler's completion tracking."""
    lane_q = {}
    for blk in nc.m.functions[0].blocks:
        for ins in blk.instructions:
            t = type(ins).__name__
            if t not in ("InstDMAGatherAnt", "InstDMAScatterAddAnt"):
                continue
            proc = getattr(ins, "bass_scheduled_proc", None)
            qn = getattr(ins, "queue_num", 0)
            if proc is None:
                continue
            lane = proc
            prev = lane_q.setdefault(lane, qn)
            assert prev == qn, (
                f"lane {lane} updated from queues {prev} and {qn}"
            )


# -------------------------------------------------------------------- kernel

def kernel(**inputs):
    global _last_results
    S_list, nh_list, has_bias, in_maps = _prep(inputs)
    key = (S_list, nh_list, has_bias)
    if key not in _cache:
        _cache[key] = _build(list(S_list), list(nh_list), has_bias)
    nc = _cache[key]
    res = bass_utils.run_bass_kernel_spmd(nc, in_maps, list(range(N_CORES)))
    _last_results = res
    out = np.zeros((N_SEQ, MAX_LEN, H_DIM), dtype=np.float32)
    for s in range(N_CORES):
        out[s] = res.results[s]["h"][0:MAX_LEN].astype(np.float32)
    out[:, 0, :] = 0.0
    return out
